# revision 3
# baseline (speedup 1.0000x reference)
"""MoE layer kernel for Trainium2, expert-parallel sparse top-2 routing.

Strategy:
  - Host (sharding): compute router top-2 per token in numpy, build one
    shard per expert e = tokens whose top-2 includes e (top1 tokens first,
    then top2 tokens), pad to common capacity C. Pass each core its
    expert's gathered tokens x^T [D, C] (bf16), its expert weights
    w1 [D, H], w2 [H, D] (bf16), replicated gate weights, a one-hot
    column selector and a validity mask.
  - Device (per core, all compute): router logits = x @ gate_w^T (PE),
    softmax (ACT/DVE), gate = p_e / (p_top1 + p_top2), aux-loss partials
    (counts + prob sums over device-top1 tokens, via PE reduction),
    hT = relu(w1^T-tiled matmul), out = (h @ w2) * gate, streamed to DRAM.
  - Host (unshard): scatter top1 rows and top2 rows into two buffers
    (no collisions), add; combine aux partials.

Numerics: matmuls in bf16 (fp32 PSUM accumulate), softmax/gates/aux in
fp32. Selection indices come from host fp32 routing; gate values from
device. Error vs fp32 reference ~1e-3 relative.
"""

import numpy as np
import ml_dtypes

B, S, D, H, E, TOPK = 4, 2048, 1024, 2048, 8, 2
N = B * S
P = 128
N_CORES = 8
CHUNK = 1024  # tokens per processing chunk

_BIR_PATCH_DONE = False


def _install_bir_patch():
    """Walrus in this container rejects >1 sync wait on CTRL instructions
    (the Tile kernel-tail Drain). Split excess waits onto NoOps."""
    global _BIR_PATCH_DONE
    if _BIR_PATCH_DONE:
        return
    import orjson
    import concourse.bass2jax as b2j
    import concourse.bass_utils as bu

    orig = bu.compile_bir_kernel

    def _legalize(bir):
        for fn in bir.get("functions", []):
            for bb in fn.get("blocks", []):
                new_insts = []
                for inst in bb.get("instructions", []):
                    si = inst.get("sync_info")
                    waits = (si or {}).get("on_wait", [])
                    if len(waits) > 1:
                        extra, keep = waits[:-1], waits[-1:]
                        si["on_wait"] = keep
                        for k, w in enumerate(extra):
                            new_insts.append(
                                {
                                    "engine": inst["engine"],
                                    "ins": [],
                                    "outs": [],
                                    "name": f"{inst['name']}_xw{k}",
                                    "opcode": "NoOp",
                                    "sync_info": {"on_update": [], "on_wait": [w]},
                                }
                            )
                    new_insts.append(inst)
                bb["instructions"] = new_insts
        return bir

    def patched(ant_bir_str, compile_dir_path, neff_name="file.neff", **kw):
        bir = _legalize(orjson.loads(ant_bir_str))
        return orig(orjson.dumps(bir), compile_dir_path, neff_name=neff_name, **kw)

    bu.compile_bir_kernel = patched
    b2j.compile_bir_kernel = patched
    _BIR_PATCH_DONE = True


def build_bass(C, repeat=None):
    """Build the SPMD bass kernel for capacity C (multiple of 128).

    repeat: if an int, wrap the whole body in a hardware loop executing it
    that many times (for wall-clock HW timing); grading uses repeat=None.
    """
    import concourse.bass as bass
    import concourse.mybir as mybir
    import concourse.tile as tile

    assert C % P == 0
    NT = C // P  # token tiles
    f32 = mybir.dt.float32
    bf16 = mybir.dt.bfloat16

    nc = bass.Bass()
    xT_d = nc.dram_tensor("xT", [D, C], bf16, kind="ExternalInput")
    w1_d = nc.dram_tensor("w1", [D, H], bf16, kind="ExternalInput")
    w2_d = nc.dram_tensor("w2", [H, D], bf16, kind="ExternalInput")
    gwT_d = nc.dram_tensor("gwT", [D, E], bf16, kind="ExternalInput")
    oneh_d = nc.dram_tensor("oneh", [P, E], f32, kind="ExternalInput")
    valid_d = nc.dram_tensor("valid", [P, NT], f32, kind="ExternalInput")
    out_d = nc.dram_tensor("out", [C, D], f32, kind="ExternalOutput")
    aux_d = nc.dram_tensor("aux", [1, E + 1], f32, kind="ExternalOutput")

    xT_r = xT_d.rearrange("(kt p) c -> kt p c", p=P)  # [8, 128, C]
    w1_r = w1_d.rearrange("(kt p) h -> kt p h", p=P)  # [8, 128, H]
    w2_r = w2_d.rearrange("(kt p) d -> kt p d", p=P)  # [16, 128, D]
    gwT_r = gwT_d.rearrange("(kt p) e -> kt p e", p=P)  # [8, 128, E]
    out_r = out_d.rearrange("(t p) d -> t p d", p=P)  # [NT, 128, D]

    KD = D // P  # 8 k-tiles over embedding dim
    KH = H // P  # 16 k-tiles over hidden dim
    MH = H // P  # 16 m-tiles of hidden (FFN1 output tiles)

    # token chunks
    chunks = []
    c0 = 0
    while c0 < C:
        cw = min(CHUNK, C - c0)
        chunks.append((c0, cw))
        c0 += cw

    with tile.TileContext(nc) as tc:
        import contextlib

        with contextlib.ExitStack() as ctx:
            resident = ctx.enter_context(tc.tile_pool(name="resident", bufs=1))
            xpool = ctx.enter_context(tc.tile_pool(name="xpool", bufs=2))
            hpool = ctx.enter_context(tc.tile_pool(name="hpool", bufs=1))
            work = ctx.enter_context(tc.tile_pool(name="work", bufs=3))
            obuf = ctx.enter_context(tc.tile_pool(name="obuf", bufs=3))
            psum = ctx.enter_context(tc.tile_pool(name="psum", bufs=1, space="PSUM"))
            psum2 = ctx.enter_context(tc.tile_pool(name="psum2", bufs=1, space="PSUM"))

            def body():
                # ---- resident loads ----
                w1s = []
                for k in range(KD):
                    t = resident.tile([P, H], bf16, tag=f"w1_{k}", name=f"w1s_{k}")
                    nc.sync.dma_start(t[:], w1_r[k])
                    w1s.append(t)
                w2s = []
                for k in range(KH):
                    t = resident.tile([P, D], bf16, tag=f"w2_{k}", name=f"w2s_{k}")
                    nc.sync.dma_start(t[:], w2_r[k])
                    w2s.append(t)
                gwts = []
                for k in range(KD):
                    t = resident.tile([P, E], bf16, tag=f"gw_{k}", name=f"gws_{k}")
                    nc.sync.dma_start(t[:], gwT_r[k])
                    gwts.append(t)
                oneh_t = resident.tile([P, E], f32, tag="oneh")
                nc.sync.dma_start(oneh_t[:], oneh_d[:])
                valid_t = resident.tile([P, NT], f32, tag="valid")
                nc.sync.dma_start(valid_t[:], valid_d[:])

                gates = resident.tile([P, NT], f32, tag="gates")
                aux_ps = psum.tile([1, E + 1], f32, tag="aux")

                for ci, (c0, cw) in enumerate(chunks):
                    nt = cw // P  # token tiles in this chunk
                    # n-slices for FFN1 moving operand
                    nsl = []
                    s0 = 0
                    while s0 < cw:
                        w = min(512, cw - s0)
                        nsl.append((s0, w))
                        s0 += w

                    # ---- load chunk tokens ----
                    xts = []
                    for k in range(KD):
                        t = xpool.tile([P, cw], bf16, tag=f"xt{k}", name=f"xt_{k}")
                        nc.sync.dma_start(t[:], xT_r[k, :, c0 : c0 + cw])
                        xts.append(t)

                    # ---- router ----
                    for t in range(nt):
                        gt = c0 // P + t  # global token tile index
                        lg = psum2.tile([P, E], f32, tag="router")
                        for k in range(KD):
                            nc.tensor.matmul(
                                lg[:],
                                lhsT=xts[k][:, t * P : (t + 1) * P],
                                rhs=gwts[k][:],
                                start=(k == 0),
                                stop=(k == KD - 1),
                            )
                        mx = work.tile([P, 1], f32, tag="mx")
                        nc.vector.reduce_max(mx[:], lg[:], axis=mybir.AxisListType.X)
                        nmx = work.tile([P, 1], f32, tag="nmx")
                        nc.scalar.mul(nmx[:], mx[:], -1.0)
                        pu = work.tile([P, E], f32, tag="pu")
                        nc.scalar.activation(
                            pu[:], lg[:], mybir.ActivationFunctionType.Exp, bias=nmx[:]
                        )
                        den = work.tile([P, 1], f32, tag="den")
                        nc.vector.reduce_sum(den[:], pu[:], axis=mybir.AxisListType.X)
                        rden = work.tile([P, 1], f32, tag="rden")
                        nc.vector.reciprocal(rden[:], den[:])
                        # normalized probs (+ ones column) for aux
                        pext = work.tile([P, E + 1], f32, tag="pext")
                        nc.vector.tensor_scalar_mul(pext[:, 0:E], pu[:], rden[:])
                        nc.vector.memset(pext[:, E : E + 1], 1.0)
                        # p_sel = prob mass of this core's expert (unnormalized)
                        prod = work.tile([P, E], f32, tag="prod")
                        nc.vector.tensor_mul(prod[:], pu[:], oneh_t[:])
                        psel = work.tile([P, 1], f32, tag="psel")
                        nc.vector.reduce_sum(psel[:], prod[:], axis=mybir.AxisListType.X)
                        # top-8 descending -> top1, top2
                        m8 = work.tile([P, 8], f32, tag="m8")
                        nc.vector.max(m8[:], pu[:])
                        den2 = work.tile([P, 1], f32, tag="den2")
                        nc.vector.tensor_add(den2[:], m8[:, 0:1], m8[:, 1:2])
                        rden2 = work.tile([P, 1], f32, tag="rden2")
                        nc.vector.reciprocal(rden2[:], den2[:])
                        g0 = work.tile([P, 1], f32, tag="g0")
                        nc.vector.tensor_mul(g0[:], psel[:], rden2[:])
                        nc.vector.tensor_mul(
                            gates[:, gt : gt + 1], g0[:], valid_t[:, gt : gt + 1]
                        )
                        # aux partials: rows where this expert is device-top1
                        ind = work.tile([P, 1], f32, tag="ind")
                        nc.vector.tensor_tensor(
                            ind[:], psel[:], m8[:, 0:1], op=mybir.AluOpType.is_equal
                        )
                        waux = work.tile([P, 1], f32, tag="waux")
                        nc.vector.tensor_mul(
                            waux[:], ind[:], valid_t[:, gt : gt + 1]
                        )
                        is_first = ci == 0 and t == 0
                        is_last = (ci == len(chunks) - 1) and (t == nt - 1)
                        nc.tensor.matmul(
                            aux_ps[:],
                            lhsT=waux[:],
                            rhs=pext[:],
                            start=is_first,
                            stop=is_last,
                        )

                    # ---- FFN1: hT[m] = relu(w1[:,m]^T @ x) over chunk ----
                    hts = []
                    for m in range(MH):
                        t = hpool.tile([P, cw], bf16, tag=f"ht{m}", name=f"ht_{m}")
                        hts.append(t)
                    for m in range(MH):
                        hps = [
                            psum2.tile([P, w], f32, tag=f"ffn1_{idx}", name=f"ffn1ps_{idx}")
                            for idx, (s0, w) in enumerate(nsl)
                        ]
                        for k in range(KD):
                            for idx, (s0, w) in enumerate(nsl):
                                nc.tensor.matmul(
                                    hps[idx][:],
                                    lhsT=w1s[k][:, m * P : (m + 1) * P],
                                    rhs=xts[k][:, s0 : s0 + w],
                                    start=(k == 0),
                                    stop=(k == KD - 1),
                                )
                        for idx, (s0, w) in enumerate(nsl):
                            nc.scalar.activation(
                                hts[m][:, s0 : s0 + w],
                                hps[idx][:],
                                mybir.ActivationFunctionType.Relu,
                            )

                    # ---- FFN2: out[t] = (h @ w2) * gate ----
                    for t in range(nt):
                        gt = c0 // P + t
                        ob = obuf.tile([P, D], f32, tag="ob")
                        for nn in range(D // 512):
                            op = psum2.tile([P, 512], f32, tag=f"ffn2_{nn}")
                            for k in range(KH):
                                nc.tensor.matmul(
                                    op[:],
                                    lhsT=hts[k][:, t * P : (t + 1) * P],
                                    rhs=w2s[k][:, nn * 512 : (nn + 1) * 512],
                                    start=(k == 0),
                                    stop=(k == KH - 1),
                                )
                            nc.scalar.activation(
                                ob[:, nn * 512 : (nn + 1) * 512],
                                op[:],
                                mybir.ActivationFunctionType.Copy,
                                scale=gates[:, gt : gt + 1],
                            )
                        nc.sync.dma_start(out_r[c0 // P + t], ob[:])

                # ---- aux out ----
                aux_sb = work.tile([1, E + 1], f32, tag="aux_sb")
                nc.scalar.copy(aux_sb[:], aux_ps[:])
                nc.sync.dma_start(aux_d[:], aux_sb[:])

            if repeat is not None:
                with tc.For_i(0, int(repeat), 1):
                    body()
            else:
                body()

    return nc


def _host_route(x, gate_w):
    """Host routing: top-2 expert ids per token (fp32, matches jax top_k
    tie-breaking by stable order)."""
    xf = np.ascontiguousarray(np.asarray(x, dtype=np.float32).reshape(N, D))
    gw = np.asarray(gate_w, dtype=np.float32)
    logits = xf @ gw.T  # [N, E]
    order = np.argsort(-logits, axis=1, kind="stable")
    top2 = order[:, :TOPK]
    return xf, logits, top2


def make_shards(x, gate_w, w1, w2):
    """Build per-core input maps + metadata for unsharding."""
    xf, _logits, top2 = _host_route(x, gate_w)
    bf = ml_dtypes.bfloat16
    gwT = np.ascontiguousarray(np.asarray(gate_w, np.float32).T.astype(bf))

    idx_a = [np.where(top2[:, 0] == e)[0] for e in range(E)]
    idx_b = [np.where(top2[:, 1] == e)[0] for e in range(E)]
    counts = [len(a) + len(b) for a, b in zip(idx_a, idx_b)]
    C = ((max(counts) + P - 1) // P) * P
    NT = C // P

    in_maps = []
    meta = []
    w1_np = np.asarray(w1, np.float32)
    w2_np = np.asarray(w2, np.float32)
    for e in range(E):
        idx = np.concatenate([idx_a[e], idx_b[e]])
        cnt = len(idx)
        idx_pad = np.concatenate([idx, np.zeros(C - cnt, np.int64)])
        xg = xf[idx_pad]  # [C, D] f32
        xT = np.ascontiguousarray(xg.T.astype(bf))  # [D, C]
        valid = (np.arange(C) < cnt).astype(np.float32)
        valid_t = np.ascontiguousarray(valid.reshape(NT, P).T)  # [P, NT]
        oneh = np.zeros((P, E), np.float32)
        oneh[:, e] = 1.0
        in_maps.append(
            {
                "xT": xT,
                "w1": np.ascontiguousarray(w1_np[e].astype(bf)),
                "w2": np.ascontiguousarray(w2_np[e].astype(bf)),
                "gwT": gwT,
                "oneh": oneh,
                "valid": valid_t,
            }
        )
        meta.append((idx_a[e], idx_b[e], cnt))
    return in_maps, meta, C


def combine(results, meta):
    """results: list per core of dicts with 'out' [C,D] f32, 'aux' [1,E+1]."""
    buf1 = np.zeros((N, D), np.float32)
    buf2 = np.zeros((N, D), np.float32)
    P_sum = np.zeros(E, np.float64)
    f_cnt = np.zeros(E, np.float64)
    for e in range(E):
        a_idx, b_idx, cnt = meta[e]
        r = np.asarray(results[e]["out"], np.float32)
        na = len(a_idx)
        buf1[a_idx] = r[:na]
        buf2[b_idx] = r[na:cnt]
        aux = np.asarray(results[e]["aux"], np.float64).reshape(-1)
        P_sum += aux[:E]
        f_cnt[e] = aux[E]
    output = (buf1 + buf2).reshape(B, S, D)
    f = f_cnt / N
    P_mean = P_sum / N
    aux_loss = np.float32(0.01 * E * np.sum(f * P_mean))
    return output, aux_loss


def emulate_core(in_map, C):
    """Numpy mirror of the device computation (approximate: fp32 math on
    bf16-rounded inputs; device bf16 matmuls differ ~1e-3)."""
    xT = np.asarray(in_map["xT"], np.float32)  # [D, C]
    w1 = np.asarray(in_map["w1"], np.float32)
    w2 = np.asarray(in_map["w2"], np.float32)
    gwT = np.asarray(in_map["gwT"], np.float32)
    oneh = np.asarray(in_map["oneh"], np.float32)[0]  # [E]
    NT = C // P
    valid = np.asarray(in_map["valid"], np.float32).T.reshape(NT * P)  # [C]

    x = xT.T  # [C, D]
    logits = x @ gwT  # [C, E]
    mx = logits.max(axis=1, keepdims=True)
    pu = np.exp(logits - mx)
    den = pu.sum(axis=1, keepdims=True)
    probs = pu / den
    psel = (pu * oneh).sum(axis=1)
    srt = np.sort(pu, axis=1)[:, ::-1]
    den2 = srt[:, 0] + srt[:, 1]
    gate = psel / den2 * valid
    ind = (psel == srt[:, 0]).astype(np.float32) * valid
    aux = np.zeros((1, E + 1), np.float32)
    aux[0, :E] = (probs * ind[:, None]).sum(axis=0)
    aux[0, E] = ind.sum()
    h = np.maximum(x @ w1, 0.0)
    out = (h @ w2) * gate[:, None]
    return {"out": out.astype(np.float32), "aux": aux}


def kernel(x, gate_w, w1, w2):
    _install_bir_patch()
    from concourse.bass_utils import run_bass_kernel_spmd

    in_maps, meta, C = make_shards(x, gate_w, w1, w2)
    nc = build_bass(C)
    res = run_bass_kernel_spmd(nc, in_maps, core_ids=list(range(N_CORES)))
    results = res.results
    return combine(results, meta)


# revision 15
# speedup vs baseline: 1.0680x; 1.0680x over previous
"""MoE layer kernel for Trainium2, expert-parallel sparse top-2 routing.

Strategy:
  - Host (sharding): compute router top-2 per token in numpy, build one
    shard per expert e = tokens whose top-2 includes e (top1 tokens first,
    then top2 tokens), pad to common capacity C. Pass each core its
    expert's gathered tokens x^T [D, C] (bf16), its expert weights
    w1 [D, H], w2 [H, D] (bf16), replicated gate weights, a one-hot
    column selector and a validity mask.
  - Device (per core, all compute): router logits = x @ gate_w^T (PE),
    softmax (ACT/DVE), gate = p_e / (p_top1 + p_top2), aux-loss partials
    (counts + prob sums over device-top1 tokens, via PE reduction),
    hT = relu(w1^T-tiled matmul), out = (h @ w2) * gate, streamed to DRAM.
  - Host (unshard): scatter top1 rows and top2 rows into two buffers
    (no collisions), add; combine aux partials.

Numerics: matmuls in bf16 (fp32 PSUM accumulate), softmax/gates/aux in
fp32. Selection indices come from host fp32 routing; gate values from
device. Error vs fp32 reference ~1e-3 relative.
"""

import numpy as np
import ml_dtypes

B, S, D, H, E, TOPK = 4, 2048, 1024, 2048, 8, 2
N = B * S
P = 128
N_CORES = 8
CHUNK = 1024  # tokens per processing chunk

_BIR_PATCH_DONE = False


def _install_bir_patch():
    """Walrus in this container rejects >1 sync wait on CTRL instructions
    (the Tile kernel-tail Drain). Split excess waits onto NoOps."""
    global _BIR_PATCH_DONE
    if _BIR_PATCH_DONE:
        return
    import orjson
    import concourse.bass2jax as b2j
    import concourse.bass_utils as bu

    orig = bu.compile_bir_kernel

    def _legalize(bir):
        for fn in bir.get("functions", []):
            for bb in fn.get("blocks", []):
                new_insts = []
                for inst in bb.get("instructions", []):
                    si = inst.get("sync_info")
                    waits = (si or {}).get("on_wait", [])
                    if len(waits) > 1:
                        extra, keep = waits[:-1], waits[-1:]
                        si["on_wait"] = keep
                        for k, w in enumerate(extra):
                            new_insts.append(
                                {
                                    "engine": inst["engine"],
                                    "ins": [],
                                    "outs": [],
                                    "name": f"{inst['name']}_xw{k}",
                                    "opcode": "NoOp",
                                    "sync_info": {"on_update": [], "on_wait": [w]},
                                }
                            )
                    new_insts.append(inst)
                bb["instructions"] = new_insts
        return bir

    def patched(ant_bir_str, compile_dir_path, neff_name="file.neff", **kw):
        bir = _legalize(orjson.loads(ant_bir_str))
        return orig(orjson.dumps(bir), compile_dir_path, neff_name=neff_name, **kw)

    bu.compile_bir_kernel = patched
    b2j.compile_bir_kernel = patched
    _BIR_PATCH_DONE = True


def build_bass(C, repeat=None, chunk=None, ffn1_bufs=2, xpool_bufs=2,
               do_router=True, do_ffn1=True, do_ffn2=True, early_weights=False,
               relu_on_dve=False):
    """Build the SPMD bass kernel for capacity C (multiple of 128).

    repeat: if an int, wrap the whole body in a hardware loop executing it
    that many times (for wall-clock HW timing); grading uses repeat=None.
    """
    import concourse.bass as bass
    import concourse.mybir as mybir
    import concourse.tile as tile

    assert C % P == 0
    if chunk is None:
        chunk = CHUNK
    NT = C // P  # token tiles
    f32 = mybir.dt.float32
    bf16 = mybir.dt.bfloat16

    nc = bass.Bass()
    xT_d = nc.dram_tensor("xT", [D, C], bf16, kind="ExternalInput")
    w1_d = nc.dram_tensor("w1", [D, H], bf16, kind="ExternalInput")
    w2_d = nc.dram_tensor("w2", [H, D], bf16, kind="ExternalInput")
    gwT_d = nc.dram_tensor("gwT", [D, E], bf16, kind="ExternalInput")
    oneh_d = nc.dram_tensor("oneh", [P, E], f32, kind="ExternalInput")
    valid_d = nc.dram_tensor("valid", [P, NT], f32, kind="ExternalInput")
    out_d = nc.dram_tensor("out", [C, D], f32, kind="ExternalOutput")
    aux_d = nc.dram_tensor("aux", [1, E + 1], f32, kind="ExternalOutput")

    xT_r = xT_d.rearrange("(kt p) c -> kt p c", p=P)  # [8, 128, C]
    w1_r = w1_d.rearrange("(kt p) h -> kt p h", p=P)  # [8, 128, H]
    w2_r = w2_d.rearrange("(kt p) d -> kt p d", p=P)  # [16, 128, D]
    gwT_r = gwT_d.rearrange("(kt p) e -> kt p e", p=P)  # [8, 128, E]
    out_r = out_d.rearrange("(t p) d -> t p d", p=P)  # [NT, 128, D]

    KD = D // P  # 8 k-tiles over embedding dim
    KH = H // P  # 16 k-tiles over hidden dim
    MH = H // P  # 16 m-tiles of hidden (FFN1 output tiles)

    # token chunks
    chunks = []
    c0 = 0
    while c0 < C:
        cw = min(chunk, C - c0)
        chunks.append((c0, cw))
        c0 += cw

    with tile.TileContext(nc) as tc:
        import contextlib

        with contextlib.ExitStack() as ctx:
            resident = ctx.enter_context(tc.tile_pool(name="resident", bufs=1))
            xpool = ctx.enter_context(tc.tile_pool(name="xpool", bufs=xpool_bufs))
            hpool = ctx.enter_context(tc.tile_pool(name="hpool", bufs=1))
            work = ctx.enter_context(tc.tile_pool(name="work", bufs=3))
            obuf = ctx.enter_context(tc.tile_pool(name="obuf", bufs=3))
            psum = ctx.enter_context(tc.tile_pool(name="psum", bufs=1, space="PSUM"))
            psum2 = ctx.enter_context(tc.tile_pool(name="psum2", bufs=1, space="PSUM"))

            def body():
                # ---- resident loads (order = consumption order: router needs
                # gw + first chunk tokens, FFN1 needs w1, FFN2 needs w2) ----
                gwts = []
                for k in range(KD):
                    t = resident.tile([P, E], bf16, tag=f"gw_{k}", name=f"gws_{k}")
                    nc.sync.dma_start(t[:], gwT_r[k])
                    gwts.append(t)
                oneh_t = resident.tile([P, E], f32, tag="oneh")
                nc.sync.dma_start(oneh_t[:], oneh_d[:])
                valid_t = resident.tile([P, NT], f32, tag="valid")
                nc.sync.dma_start(valid_t[:], valid_d[:])
                w1s = []
                for k in range(KD):
                    t = resident.tile([P, H], bf16, tag=f"w1_{k}", name=f"w1s_{k}")
                    if early_weights:
                        nc.sync.dma_start(t[:], w1_r[k])
                    w1s.append(t)
                w2s = []
                for k in range(KH):
                    t = resident.tile([P, D], bf16, tag=f"w2_{k}", name=f"w2s_{k}")
                    if early_weights:
                        nc.sync.dma_start(t[:], w2_r[k])
                    w2s.append(t)

                gates = resident.tile([P, NT], f32, tag="gates")
                aux_ps = psum.tile([1, E + 1], f32, tag="aux")

                for ci, (c0, cw) in enumerate(chunks):
                    nt = cw // P  # token tiles in this chunk
                    # n-slices for FFN1 moving operand
                    nsl = []
                    s0 = 0
                    while s0 < cw:
                        w = min(512, cw - s0)
                        nsl.append((s0, w))
                        s0 += w

                    # ---- load chunk tokens (k-interleaved with w1 on the
                    # first chunk so FFN1 starts after the first k-tiles land) ----
                    xts = []
                    for k in range(KD):
                        t = xpool.tile([P, cw], bf16, tag=f"xt{k}", name=f"xt_{k}")
                        nc.sync.dma_start(t[:], xT_r[k, :, c0 : c0 + cw])
                        xts.append(t)
                        if ci == 0 and not early_weights:
                            nc.sync.dma_start(w1s[k][:], w1_r[k])
                    if ci == 0 and not early_weights:
                        for k in range(KH):
                            nc.sync.dma_start(w2s[k][:], w2_r[k])

                    # ---- router ----
                    def router_tile(t):
                        gt = c0 // P + t  # global token tile index
                        lg = psum2.tile([P, E], f32, tag="router", name="lg")
                        for k in range(KD):
                            nc.tensor.matmul(
                                lg[:],
                                lhsT=xts[k][:, t * P : (t + 1) * P],
                                rhs=gwts[k][:],
                                start=(k == 0),
                                stop=(k == KD - 1),
                            )
                        mx = work.tile([P, 1], f32, tag="mx")
                        nc.vector.reduce_max(mx[:], lg[:], axis=mybir.AxisListType.X)
                        nmx = work.tile([P, 1], f32, tag="nmx")
                        nc.scalar.mul(nmx[:], mx[:], -1.0)
                        pu = work.tile([P, E], f32, tag="pu")
                        nc.scalar.activation(
                            pu[:], lg[:], mybir.ActivationFunctionType.Exp, bias=nmx[:]
                        )
                        den = work.tile([P, 1], f32, tag="den")
                        nc.vector.reduce_sum(den[:], pu[:], axis=mybir.AxisListType.X)
                        rden = work.tile([P, 1], f32, tag="rden")
                        nc.vector.reciprocal(rden[:], den[:])
                        # normalized probs (+ ones column) for aux
                        pext = work.tile([P, E + 1], f32, tag="pext")
                        nc.vector.tensor_scalar_mul(pext[:, 0:E], pu[:], rden[:])
                        nc.vector.memset(pext[:, E : E + 1], 1.0)
                        # p_sel = prob mass of this core's expert (unnormalized)
                        prod = work.tile([P, E], f32, tag="prod")
                        nc.vector.tensor_mul(prod[:], pu[:], oneh_t[:])
                        psel = work.tile([P, 1], f32, tag="psel")
                        nc.vector.reduce_sum(psel[:], prod[:], axis=mybir.AxisListType.X)
                        # top-8 descending -> top1, top2
                        m8 = work.tile([P, 8], f32, tag="m8")
                        nc.vector.max(m8[:], pu[:])
                        den2 = work.tile([P, 1], f32, tag="den2")
                        nc.vector.tensor_add(den2[:], m8[:, 0:1], m8[:, 1:2])
                        rden2 = work.tile([P, 1], f32, tag="rden2")
                        nc.vector.reciprocal(rden2[:], den2[:])
                        g0 = work.tile([P, 1], f32, tag="g0")
                        nc.vector.tensor_mul(g0[:], psel[:], rden2[:])
                        nc.vector.tensor_mul(
                            gates[:, gt : gt + 1], g0[:], valid_t[:, gt : gt + 1]
                        )
                        # aux partials: rows where this expert is device-top1
                        ind = work.tile([P, 1], f32, tag="ind")
                        nc.vector.tensor_tensor(
                            ind[:], psel[:], m8[:, 0:1], op=mybir.AluOpType.is_equal
                        )
                        waux = work.tile([P, 1], f32, tag="waux")
                        nc.vector.tensor_mul(
                            waux[:], ind[:], valid_t[:, gt : gt + 1]
                        )
                        is_first = ci == 0 and t == 0
                        is_last = (ci == len(chunks) - 1) and (t == nt - 1)
                        nc.tensor.matmul(
                            aux_ps[:],
                            lhsT=waux[:],
                            rhs=pext[:],
                            start=is_first,
                            stop=is_last,
                        )

                    # ---- FFN1: hT[m] = relu(w1[:,m]^T @ x) over chunk ----
                    hts = []
                    for m in range(MH):
                        t = hpool.tile([P, cw], bf16, tag=f"ht{m}", name=f"ht_{m}")
                        hts.append(t)
                    for t in range(nt) if do_router else []:
                        router_tile(t)
                    for m in range(MH) if do_ffn1 else []:
                        hps = [
                            psum2.tile([P, w], f32, tag=f"ffn1_{idx}", name=f"ffn1ps_{idx}", bufs=ffn1_bufs)
                            for idx, (s0, w) in enumerate(nsl)
                        ]
                        for k in range(KD):
                            for idx, (s0, w) in enumerate(nsl):
                                nc.tensor.matmul(
                                    hps[idx][:],
                                    lhsT=w1s[k][:, m * P : (m + 1) * P],
                                    rhs=xts[k][:, s0 : s0 + w],
                                    start=(k == 0),
                                    stop=(k == KD - 1),
                                )
                        for idx, (s0, w) in enumerate(nsl):
                            if relu_on_dve:
                                nc.vector.tensor_scalar_max(
                                    hts[m][:, s0 : s0 + w], hps[idx][:], 0.0
                                )
                            else:
                                nc.scalar.activation(
                                    hts[m][:, s0 : s0 + w],
                                    hps[idx][:],
                                    mybir.ActivationFunctionType.Relu,
                                )

                    # ---- FFN2: out[t] = (h @ w2) * gate ----
                    for t in range(nt) if do_ffn2 else []:
                        gt = c0 // P + t
                        ob = obuf.tile([P, D], f32, tag="ob")
                        for nn in range(D // 512):
                            op = psum2.tile([P, 512], f32, tag=f"ffn2_{nn}")
                            for k in range(KH):
                                nc.tensor.matmul(
                                    op[:],
                                    lhsT=hts[k][:, t * P : (t + 1) * P],
                                    rhs=w2s[k][:, nn * 512 : (nn + 1) * 512],
                                    start=(k == 0),
                                    stop=(k == KH - 1),
                                )
                            nc.scalar.activation(
                                ob[:, nn * 512 : (nn + 1) * 512],
                                op[:],
                                mybir.ActivationFunctionType.Copy,
                                scale=gates[:, gt : gt + 1],
                            )
                        nc.sync.dma_start(out_r[c0 // P + t], ob[:])

                # ---- aux out ----
                if do_router:
                    aux_sb = work.tile([1, E + 1], f32, tag="aux_sb")
                    nc.scalar.copy(aux_sb[:], aux_ps[:])
                    nc.sync.dma_start(aux_d[:], aux_sb[:])

            if repeat is not None:
                with tc.For_i(0, int(repeat), 1):
                    body()
            else:
                body()

    return nc


def _host_route(x, gate_w):
    """Host routing: top-2 expert ids per token (fp32, matches jax top_k
    tie-breaking by stable order)."""
    xf = np.ascontiguousarray(np.asarray(x, dtype=np.float32).reshape(N, D))
    gw = np.asarray(gate_w, dtype=np.float32)
    logits = xf @ gw.T  # [N, E]
    order = np.argsort(-logits, axis=1, kind="stable")
    top2 = order[:, :TOPK]
    return xf, logits, top2


def make_shards(x, gate_w, w1, w2):
    """Build per-core input maps + metadata for unsharding."""
    xf, _logits, top2 = _host_route(x, gate_w)
    bf = ml_dtypes.bfloat16
    gwT = np.ascontiguousarray(np.asarray(gate_w, np.float32).T.astype(bf))

    idx_a = [np.where(top2[:, 0] == e)[0] for e in range(E)]
    idx_b = [np.where(top2[:, 1] == e)[0] for e in range(E)]
    counts = [len(a) + len(b) for a, b in zip(idx_a, idx_b)]
    C = ((max(counts) + P - 1) // P) * P
    NT = C // P

    in_maps = []
    meta = []
    w1_np = np.asarray(w1, np.float32)
    w2_np = np.asarray(w2, np.float32)
    for e in range(E):
        idx = np.concatenate([idx_a[e], idx_b[e]])
        cnt = len(idx)
        idx_pad = np.concatenate([idx, np.zeros(C - cnt, np.int64)])
        xg = xf[idx_pad]  # [C, D] f32
        xT = np.ascontiguousarray(xg.T.astype(bf))  # [D, C]
        valid = (np.arange(C) < cnt).astype(np.float32)
        valid_t = np.ascontiguousarray(valid.reshape(NT, P).T)  # [P, NT]
        oneh = np.zeros((P, E), np.float32)
        oneh[:, e] = 1.0
        in_maps.append(
            {
                "xT": xT,
                "w1": np.ascontiguousarray(w1_np[e].astype(bf)),
                "w2": np.ascontiguousarray(w2_np[e].astype(bf)),
                "gwT": gwT,
                "oneh": oneh,
                "valid": valid_t,
            }
        )
        meta.append((idx_a[e], idx_b[e], cnt))
    return in_maps, meta, C


def combine(results, meta):
    """results: list per core of dicts with 'out' [C,D] f32, 'aux' [1,E+1]."""
    buf1 = np.zeros((N, D), np.float32)
    buf2 = np.zeros((N, D), np.float32)
    P_sum = np.zeros(E, np.float64)
    f_cnt = np.zeros(E, np.float64)
    for e in range(E):
        a_idx, b_idx, cnt = meta[e]
        r = np.asarray(results[e]["out"], np.float32)
        na = len(a_idx)
        buf1[a_idx] = r[:na]
        buf2[b_idx] = r[na:cnt]
        aux = np.asarray(results[e]["aux"], np.float64).reshape(-1)
        P_sum += aux[:E]
        f_cnt[e] = aux[E]
    output = (buf1 + buf2).reshape(B, S, D)
    f = f_cnt / N
    P_mean = P_sum / N
    aux_loss = np.float32(0.01 * E * np.sum(f * P_mean))
    return output, aux_loss


def emulate_core(in_map, C):
    """Numpy mirror of the device computation (approximate: fp32 math on
    bf16-rounded inputs; device bf16 matmuls differ ~1e-3)."""
    xT = np.asarray(in_map["xT"], np.float32)  # [D, C]
    w1 = np.asarray(in_map["w1"], np.float32)
    w2 = np.asarray(in_map["w2"], np.float32)
    gwT = np.asarray(in_map["gwT"], np.float32)
    oneh = np.asarray(in_map["oneh"], np.float32)[0]  # [E]
    NT = C // P
    valid = np.asarray(in_map["valid"], np.float32).T.reshape(NT * P)  # [C]

    x = xT.T  # [C, D]
    logits = x @ gwT  # [C, E]
    mx = logits.max(axis=1, keepdims=True)
    pu = np.exp(logits - mx)
    den = pu.sum(axis=1, keepdims=True)
    probs = pu / den
    psel = (pu * oneh).sum(axis=1)
    srt = np.sort(pu, axis=1)[:, ::-1]
    den2 = srt[:, 0] + srt[:, 1]
    gate = psel / den2 * valid
    ind = (psel == srt[:, 0]).astype(np.float32) * valid
    aux = np.zeros((1, E + 1), np.float32)
    aux[0, :E] = (probs * ind[:, None]).sum(axis=0)
    aux[0, E] = ind.sum()
    h = np.maximum(x @ w1, 0.0)
    out = (h @ w2) * gate[:, None]
    return {"out": out.astype(np.float32), "aux": aux}


def kernel(x, gate_w, w1, w2):
    _install_bir_patch()
    from concourse.bass_utils import run_bass_kernel_spmd

    in_maps, meta, C = make_shards(x, gate_w, w1, w2)
    nc = build_bass(C)
    res = run_bass_kernel_spmd(nc, in_maps, core_ids=list(range(N_CORES)))
    results = res.results
    return combine(results, meta)
